# revision 75
# baseline (speedup 1.0000x reference)
"""Trainium2 Bass kernel for the dual-branch agent-attention module.

Sharding: data-parallel over B=8 (one batch element per NeuronCore).

Host side (unchanged math vs the f32 baseline):
  - Effective score weights Weff_A = Wq @ blockdiag(k12) and
    Weff_B = Wkhf @ blockdiag(qa12); branch-A per-agent exp bias
    c_A = blockdiag(k12)^T @ bq.  Scalar softmax biases cancel; the
    branch-B per-agent bias is a per-column multiplier that cancels in
    its softmax; bv folds in after the xs normalize; bproj host-side.

fp8 strategy: x, attn, Weff_A, Weff_B are cast to float8e4 (weights
scaled by a power of two S so their tiny entries clear the e4m3
subnormal floor; 1/S folds into the exp's activation scale). The two
big SCORE matmuls run as fp8 DoubleRow (K=256/instr, 0.5 cyc/row,
C=384 padded to 512 with zero rows): softmax damps score-side quant
noise. The VALUE path (v, pt, xs, x_out, transposes, proj) stays
bf16: value-path quantization is NOT damped (attention averages
zero-mean values, shrinking signal and noise equally), so fp8 there
costs ~5% rel err (measured).

PSUM (8 banks): tag C = 2-bank slots x2 {stage1 768-wide t-scores,
stage2 A-score exp-pairs + 2-tile xo groups}, tag F = 1-bank slots x3
{stage1 v, stage2 transpose-pairs + proj tiles}, 1 bank xs accum.

Schedule: exp is ACT-only (~52us floor) and every PSUM drain must use
ACT or DVE (Pool cannot access PSUM, nor can DMA). Stage 2 runs a
4-deep chunk pipeline per step s: A-exps for chunk s+2, xo+normalize
for s, transposes for s-1, proj+store for s-2, so every exp/copy
round-trip is covered by >=1 step of PE work (PE gaps also cost
p-state downclock). A-scores for chunks 0-1 ride stage 1's ACT slack.
"""

import os
import sys
import numpy as np

for _p in ("/opt/trn_rl_repo", os.path.expanduser("~/.axon_site/_ro/trn_rl_repo")):
    if os.path.isdir(_p) and _p not in sys.path:
        sys.path.insert(0, _p)

import ml_dtypes

import concourse.bass as bass
import concourse.bacc as bacc
import concourse.tile as tile
from concourse import mybir
from concourse.bass_utils import run_bass_kernel_spmd
from concourse.masks import make_identity

BF16 = mybir.dt.bfloat16
F32 = mybir.dt.float32
FP8 = mybir.dt.float8e4
NPBF16 = ml_dtypes.bfloat16
NPFP8 = ml_dtypes.float8_e4m3
DR = mybir.MatmulPerfMode.DoubleRow

B, N, NA, H, D = 8, 4096, 64, 12, 32
C = H * D            # 384
C2 = 2 * C           # 768
C4 = 512             # C padded to 4 k-tiles of 128
NP = H // 2          # 6 head pairs
CH = 512             # seq chunk
NCH = N // CH        # 8
TPC = CH // 128      # 4 seq tiles per chunk
SCALE = D ** -0.5

_CACHE = {}


def _build_bass(zero_bias=False, sa_inv=1.0, sb_inv=1.0, finalize=True):
    nc = bacc.Bacc()

    # ---- DRAM I/O (score-side inputs row-padded to C4=512) ----
    xT = nc.dram_tensor("xT", [C4, N], FP8, kind="ExternalInput")
    attnT = nc.dram_tensor("attnT", [C4, N], FP8, kind="ExternalInput")
    attnTb = nc.dram_tensor("attnTb", [C, N], BF16, kind="ExternalInput")
    wbv = nc.dram_tensor("wbv", [C4, C2], FP8, kind="ExternalInput")  # weffb
    wvb = nc.dram_tensor("wvb", [C, C], BF16, kind="ExternalInput")
    wea = nc.dram_tensor("wea", [C4, C2], FP8, kind="ExternalInput")
    wproj = nc.dram_tensor("wproj", [C, C], BF16, kind="ExternalInput")
    if not zero_bias:
        cbav = nc.dram_tensor("cbav", [C2], F32, kind="ExternalInput")
        bvh = nc.dram_tensor("bvh", [2 * NP * D], F32, kind="ExternalInput")
    out = nc.dram_tensor("out", [N, C], BF16, kind="ExternalOutput")

    Exp = mybir.ActivationFunctionType.Exp

    with tile.TileContext(nc) as tc:
        with (
            tc.tile_pool(name="const", bufs=1) as const,
            tc.tile_pool(name="pt", bufs=4) as p_pt,
            tc.tile_pool(name="xon", bufs=8) as p_xon,
            tc.tile_pool(name="rc", bufs=3) as p_rc,
            tc.tile_pool(name="xot", bufs=6) as p_xot,
            tc.tile_pool(name="osb", bufs=3) as p_out,
            tc.tile_pool(name="psC", bufs=2, space="PSUM") as psC,
            tc.tile_pool(name="psF", bufs=3, space="PSUM") as psF,
            tc.tile_pool(name="psX", bufs=1, space="PSUM") as psX,
        ):
            # ---- constants ----
            at4 = const.tile([128, 4, N], FP8)
            atb = const.tile([128, 3, N], BF16)
            xt4 = const.tile([128, 4, N], FP8)
            w_bv = const.tile([128, 4, C2], FP8)
            w_v = const.tile([128, 3, C], BF16)
            w_ea = const.tile([128, 4, C2], FP8)
            w_pr = const.tile([128, 3, C], BF16)
            at_r = attnT.rearrange("(k p) s -> p k s", p=128)
            atb_r = attnTb.rearrange("(k p) s -> p k s", p=128)
            xt_r = xT.rearrange("(k p) s -> p k s", p=128)

            # at chunk 0 + the stage-1 weights first so stage 1 starts ASAP;
            # atb rides the gpsimd queue in parallel; xt0/xt1 + wea early so
            # the A-score pairs folded into stage 1 have their inputs.
            # head DMAs spread across four queues so stage 1's operands all
            # land in parallel within ~1.5us
            wbv_r = wbv.rearrange("(k p) m -> p k m", p=128)
            nc.sync.dma_start(out=w_bv[:, 0:2, :], in_=wbv_r[:, 0:2, :])
            nc.sync.dma_start(out=at4[:, :, 0:128], in_=at_r[:, :, 0:128])
            nc.sync.dma_start(out=w_bv[:, 2:4, :], in_=wbv_r[:, 2:4, :])
            nc.sync.dma_start(out=at4[:, :, 128:CH], in_=at_r[:, :, 128:CH])
            nc.gpsimd.dma_start(out=atb[:, :, 0:CH], in_=atb_r[:, :, 0:CH])
            nc.gpsimd.dma_start(out=w_v,
                                in_=wvb.rearrange("(k p) m -> p k m", p=128))
            nc.gpsimd.dma_start(out=xt4[:, :, 0:CH], in_=xt_r[:, :, 0:CH])
            nc.gpsimd.dma_start(out=w_ea,
                                in_=wea.rearrange("(k p) m -> p k m", p=128))
            nc.gpsimd.dma_start(out=xt4[:, :, CH:2 * CH], in_=xt_r[:, :, CH:2 * CH])
            # later chunks ride fewer, bigger transfers (HWDGE costs ~625ns
            # per dma_start, which saturates the queues early otherwise)
            for lo, hi in ((1, 2), (2, 3), (3, 5), (5, 8)):
                nc.sync.dma_start(out=at4[:, :, lo * CH:hi * CH],
                                  in_=at_r[:, :, lo * CH:hi * CH])
                nc.sync.dma_start(out=atb[:, :, lo * CH:hi * CH],
                                  in_=atb_r[:, :, lo * CH:hi * CH])
            nc.sync.dma_start(out=xt4[:, :, 2 * CH:5 * CH],
                              in_=xt_r[:, :, 2 * CH:5 * CH])
            nc.sync.dma_start(out=xt4[:, :, 5 * CH:],
                              in_=xt_r[:, :, 5 * CH:])
            nc.sync.dma_start(out=w_pr, in_=wproj.rearrange("(k p) m -> p k m", p=128))

            cba = None
            if not zero_bias:
                cba = const.tile([128, 6], F32)
                nc.gpsimd.dma_start(out=cba, in_=cbav.rearrange("(j p) -> p j", p=128))
                bvb = const.tile([128, NP, D], F32)
                nc.gpsimd.dma_start(
                    out=bvb[0:64],
                    in_=bass.AP(tensor=bvh[:].tensor, offset=0,
                                ap=[[0, 64], [1, NP * D]]))
                nc.gpsimd.dma_start(
                    out=bvb[64:128],
                    in_=bass.AP(tensor=bvh[:].tensor, offset=NP * D,
                                ap=[[0, 64], [1, NP * D]]))
                touch = const.tile([128, 4], F32)
                nc.vector.tensor_copy(touch[:, 0:1], cba[:, 0:1])
                nc.vector.tensor_copy(touch[:, 1:2], bvb[:, 0:1, 0])
            ident = const.tile([128, 128], BF16)
            make_identity(nc, ident)
            xs_bd = const.tile([128, 6, 66], BF16)
            nc.vector.memset(xs_bd, 0.0)
            nc.vector.memset(xs_bd[0:64, :, 32:33], 1.0)
            nc.vector.memset(xs_bd[64:128, :, 65:66], 1.0)
            zrow = const.tile([1, 396], BF16)
            nc.vector.memset(zrow, 0.0)
            pa_full = const.tile([128, 6, N], BF16)
            # v tiles: manual ping-pong so the ones columns are set once.
            # bf16 (not fp8): value-path quantization noise is NOT damped by
            # softmax averaging (values are zero-mean), fp8 here costs ~5% rel.
            v_t = const.tile([128, 2, 2, H, 33], BF16)
            nc.vector.memset(v_t[:, :, :, :, 32], 1.0)

            # ---- xs accumulator: open the PSUM bank with a zero matmul ----
            xs_acc = psX.tile([128, 6, 66], F32, tag="x")
            nc.tensor.matmul(xs_acc[:, :, :], lhsT=zrow[:, 0:128], rhs=zrow[:, 0:396],
                             start=True, stop=False, skip_group_check=True)

            def emit_apair(ca, jp):
                if zero_bias:
                    ap2 = psC.tile([128, 1024], F32, tag="C")
                    for h01 in range(2):
                        j = 2 * jp + h01
                        for pp in range(2):
                            nc.tensor.matmul(
                                ap2[:, h01 * 512:(h01 + 1) * 512],
                                lhsT=w_ea[:, 2 * pp:2 * pp + 2,
                                          j * 128:(j + 1) * 128],
                                rhs=xt4[:, 2 * pp:2 * pp + 2,
                                        ca * CH:(ca + 1) * CH],
                                perf_mode=DR,
                                start=(pp == 0), stop=(pp == 1))
                    nc.scalar.activation(
                        pa_full[:, 2 * jp:2 * jp + 2, ca * CH:(ca + 1) * CH],
                        ap2, Exp, scale=sa_inv)
                else:
                    for h01 in range(2):
                        j = 2 * jp + h01
                        ps = psC.tile([128, 1024], F32, tag="C")
                        for pp in range(2):
                            nc.tensor.matmul(
                                ps[:, 0:512],
                                lhsT=w_ea[:, 2 * pp:2 * pp + 2,
                                          j * 128:(j + 1) * 128],
                                rhs=xt4[:, 2 * pp:2 * pp + 2,
                                        ca * CH:(ca + 1) * CH],
                                perf_mode=DR, start=(pp == 0), stop=(pp == 1))
                        nc.scalar.activation(
                            pa_full[:, j, ca * CH:(ca + 1) * CH], ps[:, 0:512],
                            Exp, scale=sa_inv, bias=cba[:, j:j + 1])

            # ---- stage 1: t-scores (fp8 DR) + v (bf16), exp, xs accum ----
            # xs matmuls for tile t are emitted after tile t+1's score/v
            # matmuls so the PE never waits on the exp/copy round-trip.
            # A-score pairs for chunks 0-1 ride along in stage 1's ACT slack
            # so the stage-2 chain can start immediately.
            pts = {}

            def emit_xs(ti, last):
                pr = (ti // 2) % 2
                pt_t = pts[ti // 2]
                for j in range(6):
                    nc.tensor.matmul(
                        xs_acc[:, j, :],
                        lhsT=pt_t[:, ti % 2, j * 128:(j + 1) * 128],
                        rhs=v_t[:, pr, ti % 2, 2 * j:2 * j + 2, :],
                        start=False, stop=(last and j == 5),
                        skip_group_check=True)
                if ti % 2 == 1:
                    del pts[ti // 2]

            # v matmuls lag the score matmuls by 4 tiles: the head DMAs for
            # chunk 0's scores (wbv + at0) are half the bytes, so the PE
            # starts ~2us earlier; xs lags one tile more.
            NT = N // 128
            for ti in range(NT + 5):
                if ti < NT:
                    s0 = ti * 128
                    sc = psC.tile([128, 1024], F32, tag="C")
                    for pp in range(2):
                        lt = at4[:, 2 * pp:2 * pp + 2, s0:s0 + 128]
                        nc.tensor.matmul(sc[:, 0:512], lhsT=lt,
                                         rhs=w_bv[:, 2 * pp:2 * pp + 2, 0:512],
                                         perf_mode=DR,
                                         start=(pp == 0), stop=(pp == 1))
                        nc.tensor.matmul(sc[:, 512:768], lhsT=lt,
                                         rhs=w_bv[:, 2 * pp:2 * pp + 2, 512:768],
                                         perf_mode=DR,
                                         start=(pp == 0), stop=(pp == 1))
                    if ti % 2 == 0:
                        pts[ti // 2] = p_pt.tile([128, 2, C2], BF16, name="pt")
                    nc.scalar.activation(pts[ti // 2][:, ti % 2, :],
                                         sc[:, 0:768], Exp, scale=sb_inv)
                tv = ti - 4
                if 0 <= tv < NT:
                    s0 = tv * 128
                    vf = psF.tile([128, 512], F32, tag="F")
                    for k in range(3):
                        nc.tensor.matmul(vf[:, 0:384],
                                         lhsT=atb[:, k, s0:s0 + 128],
                                         rhs=w_v[:, k, :],
                                         start=(k == 0), stop=(k == 2))
                    nc.vector.tensor_copy(
                        v_t[:, (tv // 2) % 2, tv % 2, :, 0:32],
                        vf[:, 0:384].rearrange("p (h d) -> p h d", d=32))
                tx = ti - 5
                if 0 <= tx < NT:
                    emit_xs(tx, last=(tx == NT - 1))
                if 6 <= ti <= 21 and ti % 3 == 0:
                    k = (ti - 6) // 3        # 0..5
                    emit_apair(k // 3, k % 3)

            # ---- stage 1.5: xs normalize -> block-diag [xs | 1] tiles ----
            rec6 = p_rc.tile([128, 6], F32, tag="rec")
            nc.vector.reciprocal(rec6[0:64, :], xs_acc[0:64, :, 32])
            nc.vector.reciprocal(rec6[64:128, :], xs_acc[64:128, :, 65])
            nc.vector.tensor_mul(xs_bd[0:64, :, 0:32], xs_acc[0:64, :, 0:32],
                                 rec6[0:64, :].unsqueeze(2).to_broadcast([64, 6, 32]))
            nc.vector.tensor_mul(xs_bd[64:128, :, 33:65], xs_acc[64:128, :, 33:65],
                                 rec6[64:128, :].unsqueeze(2).to_broadcast([64, 6, 32]))
            if not zero_bias:
                nc.vector.tensor_add(xs_bd[0:64, :, 0:32], xs_bd[0:64, :, 0:32],
                                     bvb[0:64])
                nc.vector.tensor_add(xs_bd[64:128, :, 33:65], xs_bd[64:128, :, 33:65],
                                     bvb[64:128])

            # ---- stage 2: software pipeline over chunks ----
            # step s: A-scores+exp for chunk s+2, xo+normalize for chunk s,
            # transpose/proj/store (batched: all transposes ahead of all
            # projs, copies alternating DVE/ACT) for chunk s-1.
            xons = {}

            def emit_xo(ca, g):
                xo = psC.tile([128, 2, 512], F32, tag="C")
                for h01 in range(2):
                    s0 = ca * CH + (2 * g + h01) * 128
                    for j in range(6):
                        nc.tensor.matmul(
                            xo[:, h01, j * 66:(j + 1) * 66],
                            lhsT=pa_full[:, j, s0:s0 + 128],
                            rhs=xs_bd[:, j, :],
                            start=True, stop=True, skip_group_check=True)
                rc = p_rc.tile([128, 2, H], F32, tag="rc24")
                nc.vector.reciprocal(rc, xo[:, :, 32:396:33])
                xon = p_xon.tile([128, 2, H, D], BF16)
                nc.vector.tensor_mul(
                    xon,
                    xo[:, :, 0:396].rearrange("p h (g x) -> p h g x", x=33)
                    [:, :, :, 0:32],
                    rc.unsqueeze(3).to_broadcast([128, 2, H, D]))
                xons[(ca, g)] = xon

            def emit_tp(ca, g, on_act):
                # both tiles of an xo-group transpose into ONE psum bank;
                # one 768-wide bf16 copy drains them (DVE 2x mode).
                xon = xons.pop((ca, g))
                tp = psF.tile([128, 2, 384], BF16, tag="F")
                for h01 in range(2):
                    xonf = xon[:, h01].rearrange("p h d -> p (h d)")
                    for f in range(3):
                        nc.tensor.transpose(tp[:, h01, f * 128:(f + 1) * 128],
                                            xonf[:, f * 128:(f + 1) * 128],
                                            ident)
                xot = p_xot.tile([128, 2, C], BF16)
                if on_act:
                    nc.scalar.copy(xot, tp)
                else:
                    nc.vector.tensor_copy(xot, tp)
                return xot

            osbs = {}

            def emit_pr(ca, t, xot, on_act):
                # t==3 borrows the idle xs bank to loosen the F rotation;
                # output rides a per-chunk 4-tile batched DMA.
                if t == 3:
                    pr = psX.tile([128, 512], F32, name="prx", tag="x")
                else:
                    pr = psF.tile([128, 512], F32, tag="F")
                for f in range(3):
                    nc.tensor.matmul(pr[:, 0:384],
                                     lhsT=xot[:, t % 2, f * 128:(f + 1) * 128],
                                     rhs=w_pr[:, f, :],
                                     start=(f == 0), stop=(f == 2),
                                     skip_group_check=True)
                if t == 0:
                    osbs[ca] = p_out.tile([128, TPC, C], BF16, name="osb")
                o_sb = osbs[ca]
                if on_act:
                    nc.scalar.copy(o_sb[:, t, :], pr[:, 0:384])
                else:
                    nc.vector.tensor_copy(o_sb[:, t, :], pr[:, 0:384])
                if ca == NCH - 1 and t == 1:
                    # final chunk: store the first half early so the drain
                    # does not wait on one big trailing DMA
                    nc.sync.dma_start(
                        out=out[ca * CH:ca * CH + 256, :].rearrange(
                            "(t p) c -> p t c", p=128),
                        in_=o_sb[:, 0:2, :])
                if t == TPC - 1:
                    lo = 2 if ca == NCH - 1 else 0
                    q = nc.scalar if ca == NCH - 1 else nc.sync
                    q.dma_start(
                        out=out[ca * CH + lo * 128:(ca + 1) * CH, :].rearrange(
                            "(t p) c -> p t c", p=128),
                        in_=osbs.pop(ca)[:, lo:, :])

            # step s: A-exps for chunk s+2, xo+normalize for chunk s,
            # transposes+copies for chunk s-1, proj+store for chunk s-2.
            # tp->xot copies are consumed a full step later, so the PE
            # never waits on a copy round-trip.
            xts = {}
            for step in range(NCH + 3):
                ca, cx = step + 2, step
                c1, c2 = step - 1, step - 2   # transpose chunk / proj chunk
                has_a = ca < NCH
                has_x = cx < NCH
                has_1 = 0 <= c1 < NCH
                has_2 = 0 <= c2 < NCH
                # xot-pair copies on DVE (2x mode); osb copies split.
                tp_act = (False, False) if has_a else (False, True)
                pr_act = (True, False, True, False) if has_a \
                    else ((True, False, True, False) if c2 == NCH - 1
                          else (True, True, True, True))
                if step == 0:
                    # A-pairs first: they cover the stage-1.5 (xs_bd) round
                    # trip that xo(0) depends on.
                    for jp in range(3):
                        emit_apair(ca, jp)
                    emit_xo(cx, 0)
                    emit_xo(cx, 1)
                    continue
                # xo0 at step start: its rec+mul lead the DVE queue, so the
                # psC buffer it holds frees early for next step's allocs.
                if has_x:
                    emit_xo(cx, 0)
                if has_1:
                    xts[(c1, 0)] = emit_tp(c1, 0, tp_act[0])
                if has_a:
                    emit_apair(ca, 0)
                if has_2:
                    emit_pr(c2, 0, xts[(c2, 0)], pr_act[0])
                if has_a:
                    emit_apair(ca, 1)
                if has_2:
                    emit_pr(c2, 1, xts.pop((c2, 0)), pr_act[1])
                if has_1:
                    xts[(c1, 1)] = emit_tp(c1, 1, tp_act[1])
                if has_x:
                    emit_xo(cx, 1)
                if has_2:
                    emit_pr(c2, 2, xts[(c2, 1)], pr_act[2])
                if has_a:
                    emit_apair(ca, 2)
                if has_2:
                    emit_pr(c2, 3, xts.pop((c2, 1)), pr_act[3])
    if finalize:
        nc.finalize()
    return nc


def _pow2_scale(arr, target=120.0):
    m = float(np.abs(arr).max())
    if m == 0.0 or not np.isfinite(m):
        return 1.0
    k = int(np.floor(np.log2(target / m)))
    k = max(min(k, 14), -14)
    return float(2.0 ** k)


def _prep_host(inputs):
    f32 = np.float32
    x = np.asarray(inputs["x"], f32)
    attn = np.asarray(inputs["attn"], f32)
    agent = np.asarray(inputs["agent_input"], f32)
    wa = np.asarray(inputs["wa"], f32)
    wb = np.asarray(inputs["wb"], f32)

    # head-major permutation: (h, br, d) -> h*64 + br*32 + d, with the
    # branch score scales (wa/wb * D^-0.5) folded into the k-side weights
    perm = np.empty(C2, np.int64)
    sva = np.empty(C2, f32)
    svb = np.empty(C2, f32)
    for h in range(H):
        for br in range(2):
            j0 = h * 64 + br * 32
            perm[j0:j0 + 32] = br * C + h * 32 + np.arange(32)
            sva[j0:j0 + 32] = wa[br] * SCALE
            svb[j0:j0 + 32] = wb[br] * SCALE

    wq_p = np.asarray(inputs["Wq_lf"], f32)[:, perm]
    bq_p = np.asarray(inputs["bq_lf"], f32)[perm]
    wkag_p = np.asarray(inputs["Wk_ag"], f32)[:, perm] * sva[None, :]
    bkag_p = np.asarray(inputs["bk_ag"], f32)[perm] * sva
    wqag_p = np.asarray(inputs["Wq_ag"], f32)[:, perm]
    bqag_p = np.asarray(inputs["bq_ag"], f32)[perm]
    wkhf_p = np.asarray(inputs["Wk_hf"], f32)[:, perm] * svb[None, :]

    zb = all(not np.any(np.asarray(inputs[k]))
             for k in ("bq_lf", "bk_ag", "bq_ag", "bk_hf", "bv_hf", "ba", "bb"))

    shared = {}
    if not zb:
        bv_in = np.asarray(inputs["bv_hf"], f32)
        # bvh[half, j, d]: half 0 = head 2j, half 1 = head 2j+1
        bvh = np.empty((2, NP, D), f32)
        for j in range(NP):
            bvh[0, j, :] = bv_in[(2 * j) * D:(2 * j + 1) * D]
            bvh[1, j, :] = bv_in[(2 * j + 1) * D:(2 * j + 2) * D]
        shared["bvh"] = np.ascontiguousarray(bvh.reshape(-1))

    # per-batch: agent projections -> block-diag -> effective weights
    kag = agent @ wkag_p + bkag_p          # [B, 64, 768]
    qa = agent @ wqag_p + bqag_p           # [B, 64, 768]
    weffa = np.zeros((B, C, C2), f32)
    weffb = np.zeros((B, C, C2), f32)
    cba = np.zeros((B, C2), f32)
    for j in range(NP):
        j0 = j * 128
        for half, hlo in ((slice(j0, j0 + 64), slice(0, 64)),
                          (slice(j0 + 64, j0 + 128), slice(64, 128))):
            # k12_j block: [projdims 64, agents 64] = kag[:, :, half].T
            kj = kag[:, :, half].transpose(0, 2, 1)      # [B, 64p, 64a]
            qj = qa[:, :, half].transpose(0, 2, 1)
            weffa[:, :, j0:j0 + 128][:, :, hlo] = wq_p[:, half] @ kj
            weffb[:, :, j0:j0 + 128][:, :, hlo] = wkhf_p[:, half] @ qj
            cba[:, j0:j0 + 128][:, hlo] = np.einsum(
                "bpa,p->ba", kj, bq_p[half])

    # fp8 power-of-two scaling (global so all cores share one program)
    s_a = _pow2_scale(weffa)
    s_b = _pow2_scale(weffb)

    wbv4 = np.zeros((B, C4, C2), f32)
    wbv4[:, 0:C] = weffb * s_b
    wea4 = np.zeros((B, C4, C2), f32)
    wea4[:, 0:C] = weffa * s_a

    xT = np.zeros((B, C4, N), NPFP8)
    attnT = np.zeros((B, C4, N), NPFP8)
    attn_t = np.ascontiguousarray(attn.transpose(0, 2, 1))
    xT[:, 0:C] = x.transpose(0, 2, 1).astype(NPFP8)
    attnT[:, 0:C] = attn_t.astype(NPFP8)
    attnTb = attn_t.astype(NPBF16)
    wbv8 = wbv4.astype(NPFP8)
    wea8 = wea4.astype(NPFP8)
    shared["wproj"] = np.asarray(inputs["Wproj"], f32).astype(NPBF16)
    shared["wvb"] = np.asarray(inputs["Wv_hf"], f32).astype(NPBF16)

    in_maps = []
    for b in range(B):
        m = dict(shared)
        m["xT"] = np.ascontiguousarray(xT[b])
        m["attnT"] = np.ascontiguousarray(attnT[b])
        m["attnTb"] = attnTb[b]
        m["wbv"] = wbv8[b]
        m["wea"] = wea8[b]
        if not zb:
            # cbav flat layout: value for (pair j, agent-col a) at j*128+a
            m["cbav"] = np.ascontiguousarray(cba[b])
        in_maps.append(m)
    return in_maps, zb, 1.0 / s_a, 1.0 / s_b


def kernel(**inputs):
    in_maps, zb, sa_inv, sb_inv = _prep_host(inputs)
    key = ("nc", zb, sa_inv, sb_inv)
    if key not in _CACHE:
        _CACHE[key] = _build_bass(zero_bias=zb, sa_inv=sa_inv, sb_inv=sb_inv)
    nc = _CACHE[key]
    res = run_bass_kernel_spmd(nc, in_maps, core_ids=list(range(B)))
    outs = np.stack([np.asarray(res.results[b]["out"], np.float32)
                     for b in range(B)], axis=0)
    if not zb:
        outs = outs + np.asarray(inputs["bproj"], np.float32)[None, None, :]
    return outs
